# revision 1
# baseline (speedup 1.0000x reference)
"""Trainium2 Bass kernel for nn_AbsolutePE_LM (single-head causal transformer block + LM head).

Model (fp32 reference):
    h = embed[x] + pe[:C]
    Q = h Wq^T ; K = h Wk^T ; V = h Wv^T
    A = softmax(QK^T/sqrt(D) + causal)
    hidden = h + A V
    logits = hidden Wo^T + bo

Algebraic restructure (device computes fewer FLOPs; host precomputes M):
    M := Wq^T Wk                (host, fp32)
    scores = (h_q M) h_kv^T / sqrt(D)      -> no K projection on device
    attn   = (A h_kv) Wv^T                 -> no V projection on device
    hidden = h_q + attn
    logits = hidden Wo^T + bo

Sharding: 8 cores = (batch b in 0..3) x (query-half hh in 0..1).
Each core holds the full 2048-token context of its batch and computes
attention + the 1024x32000 vocab projection for its 1024 query rows.
One program on all cores; per-core behaviour is carried by the input
data (token ids, gathered positional rows, qpos/kpos vectors).

Pipelining: the query gather + Q'-projection run first; the kv gather
then streams while scores for each landed key-block run immediately
(kb-major), so the PE hides the gather DMA.

Causal trim: query-tile qt (512 queries) only loops key-blocks
kb < 12 + 4*qt. Exact for half-1 cores; for half-0 cores the
data-driven is_ge mask zeroes the extra blocks.

Precision: attention matmuls run fp8(e4m3)+DoubleRow at 2x PE rate
(attn is ~2% of hidden and softmax is near-uniform, so fp8 noise there
is ~0.1% of logits); residual h and the vocab projection in bf16;
logits emitted bf16 and upcast on host. Scales are powers of two
folded into host-side tensors: embed/pe carry 2^10, Wv^T/M carry
2^11, Wo^T carries 2^-10 so logits come out of the last matmul
unscaled.
"""

import numpy as np

V, D, MAXLEN, B, C = 32000, 1024, 2048, 4, 2048
P = 128
DH = D // P            # 8 partition tiles over the model dim
NQ = C // 2            # 1024 query rows per core
TQ = NQ // P           # 8 query row-tiles
TKV = C // P           # 16 kv row-tiles
QT = 512               # attention query-tile width
NQT = NQ // QT         # 2 attention query tiles
NKB = C // P           # 16 key blocks of 128
KBM = [12, 16]         # causal key-block bound per query tile
VT = 512               # vocab tile width
N_CORES = 8

SH = 1024.0            # 2^10 scale on h (embed/pe, host)
SW = 2048.0            # 2^11 scale on M and Wv^T (host)

_COMPILED = None


def _build_program():
    import concourse.bacc as bacc
    import concourse.mybir as mybir
    import concourse.tile as tile
    from concourse import bass
    from concourse.masks import make_identity

    f32 = mybir.dt.float32
    f32r = mybir.dt.float32r
    bf16 = mybir.dt.bfloat16
    fp8 = mybir.dt.float8e4
    i32 = mybir.dt.int32
    Exp = mybir.ActivationFunctionType.Exp
    Copy = mybir.ActivationFunctionType.Copy
    DR = mybir.MatmulPerfMode.DoubleRow

    nc = bacc.Bacc("TRN2", target_bir_lowering=False, debug=False, num_devices=N_CORES)

    hqT_d = nc.dram_tensor("hqT", [D, NQ], bf16, kind="ExternalInput").ap()
    hkvT8_d = nc.dram_tensor("hkvT8", [D, C], fp8, kind="ExternalInput").ap()
    hkv8_d = nc.dram_tensor("hkv8", [C, D], fp8, kind="ExternalInput").ap()
    wm_d = nc.dram_tensor("wm", [D, D], bf16, kind="ExternalInput").ap()
    wv_d = nc.dram_tensor("wv", [D, D], fp8, kind="ExternalInput").ap()
    woT_d = nc.dram_tensor("woT", [D, V], bf16, kind="ExternalInput").ap()
    bo_d = nc.dram_tensor("bo", [1, V], f32r, kind="ExternalInput").ap()
    nmask = sum(KBM)
    mask_d = nc.dram_tensor("mask", [P, nmask, QT], fp8, kind="ExternalInput").ap()
    y_d = nc.dram_tensor("y", [NQ, V], bf16, kind="ExternalOutput").ap()

    hqT_r = hqT_d.rearrange("(dh p) q -> p dh q", p=P)
    hkvT8_r = hkvT8_d.rearrange("(dh p) k -> p dh k", p=P)
    hkv8_r = hkv8_d.rearrange("(t p) e -> p t e", p=P)
    wm_r = wm_d.rearrange("(dh p) e -> p dh e", p=P)
    wv_r = wv_d.rearrange("(dh p) e -> p dh e", p=P)
    woT_r = woT_d.rearrange("(dh p) v -> p dh v", p=P)

    with tile.TileContext(nc) as tc:
        with tc.tile_pool(name="persist", bufs=1) as persist:
            ones8 = persist.tile([P, 2, P], fp8, tag="ones8")
            nc.gpsimd.memset(ones8[:], 1.0)
            ones1f = persist.tile([1, P], f32, tag="ones1f")
            nc.gpsimd.memset(ones1f[:], 1.0)
            ones1 = persist.tile([1, P], f32r, tag="ones1")
            nc.vector.tensor_copy(ones1[:], ones1f[:])
            hqT = persist.tile([P, DH, NQ], bf16, tag="hqT")  # becomes hiddenT in place
            nc.sync.dma_start(hqT[:], hqT_r[:])
            mask_sb = persist.tile([P, nmask, QT], fp8, tag="mask")
            nc.sync.dma_start(mask_sb[:], mask_d[:])

            # ---- Phases A-C scope ----
            with tc.tile_pool(name="kv_pool", bufs=1) as kv_pool:
                hkvT8 = kv_pool.tile([P, DH, C], fp8, tag="hkvT8")
                h_kv8 = kv_pool.tile([P, TKV, D], fp8, tag="h_kv8")
                QTs8 = kv_pool.tile([P, DH, NQ], fp8, tag="QTs8")
                wv_sb = kv_pool.tile([P, DH, D], fp8, tag="wv_sb")

                if True:
                    # ---- Phase B: Q' = h_q M (fp8 DoubleRow), eviction rescales ----
                    with tc.tile_pool(name="wload", bufs=2) as wload, \
                         tc.tile_pool(name="qp_ps", bufs=4, space="PSUM") as qp_ps:
                        for eh in range(DH):
                            wm_t = wload.tile([P, DH, P], bf16, tag="wm")
                            nc.sync.dma_start(wm_t[:], wm_r[:, :, eh * P:(eh + 1) * P])
                            for half in range(NQ // 512):
                                ps = qp_ps.tile([P, 512], f32, tag="ps")
                                for dh in range(DH):
                                    nc.tensor.matmul(
                                        ps[:],
                                        lhsT=wm_t[:, dh, :],
                                        rhs=hqT[:, dh, half * 512:(half + 1) * 512],
                                        start=(dh == 0), stop=(dh == DH - 1),
                                    )
                                nc.scalar.activation(
                                    QTs8[:, eh, half * 512:(half + 1) * 512], ps[:],
                                    Copy, scale=2.0)

                    # kv-side loads enqueue after B's wm tiles so B starts early;
                    # scores wait on these via data deps anyway
                    nc.sync.dma_start(hkvT8[:], hkvT8_r[:])
                    nc.sync.dma_start(h_kv8[:], hkv8_r[:])
                    nc.sync.dma_start(wv_sb[:], wv_r[:])

                    # ---- Phase A-kv + C-scores: kv gather with kb-major scores ----
                    with tc.tile_pool(name="att_sb", bufs=6) as att_sb, \
                         tc.tile_pool(name="expm_pool", bufs=2) as expm_pool, \
                         tc.tile_pool(name="zt_pool", bufs=2) as zt_pool, \
                         tc.tile_pool(name="dn_ps", bufs=2, space="PSUM") as dn_ps:
                        expm = []
                        den = []
                        for _qt in range(NQT):
                            expm_t = expm_pool.tile([P, NKB, QT], fp8, tag="expm_all")
                            expm.append(expm_t)
                            den_t = dn_ps.tile([P, QT], f32, tag="den")
                            den.append(den_t)
                        mi = 0
                        with tc.tile_pool(name="sc_ps", bufs=4, space="PSUM") as sc_ps:
                            for kb in range(TKV):
                                for qt in range(NQT):
                                    if kb >= KBM[qt]:
                                        continue
                                    qs = slice(qt * QT, (qt + 1) * QT)
                                    s_ps = sc_ps.tile([P, QT], f32, tag="sc")
                                    for dhp in range(0, DH, 2):
                                        nc.tensor.matmul(
                                            s_ps[:],
                                            lhsT=hkvT8[:, dhp:dhp + 2, kb * P:(kb + 1) * P],
                                            rhs=QTs8[:, dhp:dhp + 2, qs],
                                            start=(dhp == 0), stop=(dhp == DH - 2),
                                            perf_mode=DR,
                                        )
                                    expT = att_sb.tile([P, QT], fp8, tag="expT")
                                    # scores carry 2^21 (h 2^10 * Q' 2^11)
                                    nc.scalar.activation(
                                        expT[:], s_ps[:], Exp,
                                        scale=float(2.0 ** -21 / np.sqrt(D)))
                                    nc.vector.tensor_mul(
                                        expm[qt][:, kb, :], expT[:], mask_sb[:, mi, :])
                                    mi += 1
                                    if kb % 2 == 1:
                                        nc.tensor.matmul(
                                            den[qt][:],
                                            lhsT=ones8[:],
                                            rhs=expm[qt][:, kb - 1:kb + 1, :],
                                            start=(kb == 1), stop=(kb == KBM[qt] - 1),
                                            perf_mode=DR,
                                        )

                        # ---- Phase C tail: normalize, Z = A h, attn projection ----
                        with tc.tile_pool(name="zt_ps", bufs=2, space="PSUM") as zt_ps, \
                             tc.tile_pool(name="at_ps", bufs=2, space="PSUM") as at_ps:
                            for qt in range(NQT):
                                qs = slice(qt * QT, (qt + 1) * QT)
                                kbm = KBM[qt]
                                recip = att_sb.tile([P, QT], f32, tag="recip")
                                nc.vector.reciprocal(recip[:], den[qt][:])
                                ZT8 = zt_pool.tile([P, DH, QT], fp8, tag="ZT8")
                                for eh in range(DH):
                                    z_ps = zt_ps.tile([P, QT], f32, tag="z")
                                    for kbp in range(0, kbm, 2):
                                        nc.tensor.matmul(
                                            z_ps[:],
                                            lhsT=h_kv8[:, kbp:kbp + 2, eh * P:(eh + 1) * P],
                                            rhs=expm[qt][:, kbp:kbp + 2, :],
                                            start=(kbp == 0), stop=(kbp == kbm - 2),
                                            perf_mode=DR,
                                        )
                                    nc.vector.tensor_mul(ZT8[:, eh, :], z_ps[:], recip[:])
                                # attn_out^T = Wv Z^T, accumulated into hiddenT (2^10)
                                for eh in range(DH):
                                    a_ps = at_ps.tile([P, QT], f32, tag="at")
                                    for dhp in range(0, DH, 2):
                                        nc.tensor.matmul(
                                            a_ps[:],
                                            lhsT=wv_sb[:, dhp:dhp + 2, eh * P:(eh + 1) * P],
                                            rhs=ZT8[:, dhp:dhp + 2, :],
                                            start=(dhp == 0), stop=(dhp == DH - 2),
                                            perf_mode=DR,
                                        )
                                    tmp = att_sb.tile([P, QT], bf16, tag="tmp")
                                    nc.scalar.activation(tmp[:], a_ps[:], Copy,
                                                         scale=float(2.0 ** -11))
                                    nc.vector.tensor_add(hqT[:, eh, qs], hqT[:, eh, qs], tmp[:])

            # ---- Phase D: logits = hiddenT^T WoT + bo (paired vocab tiles) ----
            with tc.tile_pool(name="wo_pool", bufs=4) as wo_pool, \
                 tc.tile_pool(name="out_sb", bufs=4) as out_sb, \
                 tc.tile_pool(name="bias_sb", bufs=2) as bias_sb, \
                 tc.tile_pool(name="bo_sb", bufs=2) as bo_sb, \
                 tc.tile_pool(name="out_ps", bufs=6, space="PSUM") as out_ps, \
                 tc.tile_pool(name="bias_ps", bufs=2, space="PSUM") as bias_ps:
                nt = (V + VT - 1) // VT
                groups = []
                i = 0
                while i < nt:
                    n0 = i * VT
                    if i + 1 < nt:
                        groups.append([(n0, min(VT, V - n0)), (n0 + VT, min(VT, V - n0 - VT))])
                        i += 2
                    else:
                        groups.append([(n0, min(VT, V - n0))])
                        i += 1
                for grp in groups:
                    gw = sum(nw for _, nw in grp)
                    g0 = grp[0][0]
                    wo_c0 = wo_pool.tile([P, DH // 2, 2 * VT], bf16, tag="wo")
                    wo_c1 = wo_pool.tile([P, DH // 2, 2 * VT], bf16, tag="wo")
                    nc.sync.dma_start(wo_c0[:, :, :gw], woT_r[:, 0:DH // 2, g0:g0 + gw])
                    nc.sync.dma_start(wo_c1[:, :, :gw], woT_r[:, DH // 2:DH, g0:g0 + gw])
                    b_sb = bias_sb.tile([P, 2 * VT], f32, tag="biassb")
                    bo_t = bo_sb.tile([1, 2 * VT], f32r, tag="bo")
                    nc.sync.dma_start(bo_t[:1, :gw], bo_d[:1, g0:g0 + gw])
                    for j, (n0, nw) in enumerate(grp):
                        b_ps = bias_ps.tile([P, VT], f32, tag="bias")
                        nc.tensor.matmul(
                            b_ps[:, :nw],
                            lhsT=ones1[:1, :],
                            rhs=bo_t[:1, j * VT:j * VT + nw],
                            start=True, stop=True,
                        )
                        nc.vector.tensor_copy(b_sb[:, j * VT:j * VT + nw], b_ps[:, :nw])
                    for m in range(TQ):
                        pss = []
                        for _j in grp:
                            ps_t = out_ps.tile([P, VT], f32, tag="out")
                            pss.append(ps_t)
                        for dh in range(DH):
                            wo_t = wo_c0 if dh < DH // 2 else wo_c1
                            for j, (n0, nw) in enumerate(grp):
                                nc.tensor.matmul(
                                    pss[j][:, :nw],
                                    lhsT=hqT[:, dh, m * P:(m + 1) * P],
                                    rhs=wo_t[:, dh % (DH // 2), j * VT:j * VT + nw],
                                    start=(dh == 0), stop=(dh == DH - 1),
                                )
                        lo = out_sb.tile([P, 2 * VT], bf16, tag="lo")
                        for j, (n0, nw) in enumerate(grp):
                            nc.vector.tensor_add(
                                lo[:, j * VT:j * VT + nw], pss[j][:, :nw], b_sb[:, j * VT:j * VT + nw])
                        nc.sync.dma_start(y_d[m * P:(m + 1) * P, g0:g0 + gw], lo[:, :gw])

    nc.compile()
    return nc


def _get_program():
    global _COMPILED
    if _COMPILED is None:
        _COMPILED = _build_program()
    return _COMPILED


def kernel(x, embed, pe, Wq, Wk, Wv, Wo, bo):
    import ml_dtypes
    from concourse.bass_utils import run_bass_kernel_spmd

    bf16 = ml_dtypes.bfloat16
    fp8 = ml_dtypes.float8_e4m3fn
    x = np.asarray(x).astype(np.int32)
    embed = np.asarray(embed, dtype=np.float32)
    pe = np.asarray(pe, dtype=np.float32)
    Wq = np.asarray(Wq, dtype=np.float32)
    Wk = np.asarray(Wk, dtype=np.float32)

    h_all = (embed[x.reshape(-1)].reshape(B, C, D) + pe[None, :C, :]) * SH
    h8_all = np.clip(h_all, -240.0, 240.0).astype(fp8)
    h_all = h_all.astype(bf16)
    wm_bf = np.ascontiguousarray((Wq.T @ Wk).astype(bf16))
    wv8 = np.ascontiguousarray(
        np.clip(np.asarray(Wv, dtype=np.float32).T * SW, -240.0, 240.0).astype(fp8))
    woT = np.ascontiguousarray(
        (np.asarray(Wo, dtype=np.float32).T * (1.0 / SH)).astype(bf16))
    bo2 = np.asarray(bo, dtype=np.float32).reshape(1, V)


    nc = _get_program()

    in_maps = []
    for c in range(N_CORES):
        b, hh = c // 2, c % 2
        q0 = hh * NQ
        xb = x[b]
        # causal masks in (kb-major, qt-minor) loop order: mask[p, i, q] =
        # 1.0 if absolute query (q0 + qt*QT + q) >= absolute key (kb*P + p)
        blocks = []
        for kb in range(NKB):
            for qt in range(len(KBM)):
                if kb < KBM[qt]:
                    qpos = q0 + qt * QT + np.arange(QT)
                    kposv = kb * P + np.arange(P)
                    blocks.append(
                        (qpos[None, :] >= kposv[:, None]).astype(fp8))
        maskarr = np.ascontiguousarray(np.stack(blocks, axis=1))
        in_maps.append({
            "hqT": np.ascontiguousarray(h_all[b, q0:q0 + NQ].T),
            "hkvT8": np.ascontiguousarray(h8_all[b].T),
            "hkv8": h8_all[b],
            "wm": wm_bf,
            "wv": wv8,
            "woT": woT,
            "bo": bo2,
            "mask": maskarr,
        })

    global _last_in_maps
    _last_in_maps = in_maps
    res = run_bass_kernel_spmd(nc, in_maps, core_ids=list(range(N_CORES)))

    out = np.empty((B, C, V), dtype=np.float32)
    for c in range(N_CORES):
        b, hh = c // 2, c % 2
        out[b, hh * NQ:(hh + 1) * NQ, :] = res.results[c]["y"].astype(np.float32)
    return out



# revision 3
# speedup vs baseline: 1.2356x; 1.2356x over previous
"""Trainium2 Bass kernel for nn_AbsolutePE_LM (single-head causal transformer block + LM head).

Model (fp32 reference):
    h = embed[x] + pe[:C]
    Q = h Wq^T ; K = h Wk^T ; V = h Wv^T
    A = softmax(QK^T/sqrt(D) + causal)
    hidden = h + A V
    logits = hidden Wo^T + bo

Algebraic restructure (device computes fewer FLOPs; host precomputes M):
    M := Wq^T Wk                (host, fp32)
    scores = (h_q M) h_kv^T / sqrt(D)      -> no K projection on device
    attn   = (A h_kv) Wv^T                 -> no V projection on device
    hidden = h_q + attn
    logits = hidden Wo^T       (+ bo on host)

Sharding: 8 cores = (batch b in 0..3) x (query-half hh in 0..1).
Each core holds the full 2048-token context of its batch and computes
attention + the 1024x32000 vocab projection for its 1024 query rows.
One program on all cores; per-core behaviour is carried by the input
data (token ids, gathered positional rows, causal masks).

Schedule: all SBUF tiles coexist (no pool scope-closes mid-kernel) so
Phase D's Wo tiles prefetch during attention.  Input DMAs are ordered
so the first Q'-projection matmul starts as soon as half of hqT and
the first wm tile land.  The softmax denominator accumulation runs as
a tail pass per query tile instead of interleaved with the scores loop
(the interleaved version stalled the in-order PE behind the
Exp->mask-mul chain).  Phase D psum eviction runs on the Scalar engine
(Vector handles attention elementwise; GpSimd takes the residual add).

Precision: attention matmuls run fp8(e4m3)+DoubleRow at 2x PE rate
(attn is ~2% of hidden and softmax is near-uniform, so fp8 noise there
is ~0.1% of logits); residual h and the vocab projection in bf16;
logits emitted bf16 and upcast on host. Scales are powers of two
folded into host-side tensors: embed/pe carry 2^10, Wv^T/M carry
2^11, Wo^T carries 2^-10 so logits come out of the last matmul
unscaled.
"""

import numpy as np

V, D, MAXLEN, B, C = 32000, 1024, 2048, 4, 2048
P = 128
DH = D // P            # 8 partition tiles over the model dim
NQ = C // 2            # 1024 query rows per core
TQ = NQ // P           # 8 query row-tiles
TKV = C // P           # 16 kv row-tiles
QT = 512               # attention query-tile width
NQT = NQ // QT         # 2 attention query tiles
NKB = C // P           # 16 key blocks of 128
KBM = [12, 16]         # causal key-block bound per query tile
VT = 512               # vocab tile width
N_CORES = 8

SH = 1024.0            # 2^10 scale on h (embed/pe, host)
SW = 2048.0            # 2^11 scale on M and Wv^T (host)

_COMPILED = None


def _build_program():
    import concourse.bacc as bacc
    import concourse.mybir as mybir
    import concourse.tile as tile
    from concourse import bass
    from concourse.masks import make_identity

    f32 = mybir.dt.float32
    bf16 = mybir.dt.bfloat16
    fp8 = mybir.dt.float8e4
    Exp = mybir.ActivationFunctionType.Exp
    Copy = mybir.ActivationFunctionType.Copy
    DR = mybir.MatmulPerfMode.DoubleRow

    nc = bacc.Bacc("TRN2", target_bir_lowering=False, debug=False, num_devices=N_CORES)

    hqT_d = nc.dram_tensor("hqT", [D, NQ], bf16, kind="ExternalInput").ap()
    hkvT8_d = nc.dram_tensor("hkvT8", [D, C], fp8, kind="ExternalInput").ap()
    hkv8_d = nc.dram_tensor("hkv8", [C, D], fp8, kind="ExternalInput").ap()
    wm_d = nc.dram_tensor("wm", [D, D], bf16, kind="ExternalInput").ap()
    wv_d = nc.dram_tensor("wv", [D, D], fp8, kind="ExternalInput").ap()
    woT_d = nc.dram_tensor("woT", [D, V], bf16, kind="ExternalInput").ap()
    nmask = sum(KBM)
    mask_d = nc.dram_tensor("mask", [P, nmask, QT], fp8, kind="ExternalInput").ap()
    y_d = nc.dram_tensor("y", [NQ, V], bf16, kind="ExternalOutput").ap()

    hqT_r = hqT_d.rearrange("(dh p) q -> p dh q", p=P)
    hkvT8_r = hkvT8_d.rearrange("(dh p) k -> p dh k", p=P)
    hkv8_r = hkv8_d.rearrange("(t p) e -> p t e", p=P)
    wm_r = wm_d.rearrange("(dh p) e -> p dh e", p=P)
    wv_r = wv_d.rearrange("(dh p) e -> p dh e", p=P)
    woT_r = woT_d.rearrange("(dh p) v -> p dh v", p=P)

    with tile.TileContext(nc) as tc:
        with tc.tile_pool(name="persist", bufs=1) as persist, \
             tc.tile_pool(name="wload", bufs=2) as wload, \
             tc.tile_pool(name="att_sb", bufs=6) as att_sb, \
             tc.tile_pool(name="expm_pool", bufs=2) as expm_pool, \
             tc.tile_pool(name="zt_pool", bufs=2) as zt_pool, \
             tc.tile_pool(name="wo_pool", bufs=4) as wo_pool, \
             tc.tile_pool(name="out_sb", bufs=4) as out_sb:
            ones8 = persist.tile([P, 2, P], fp8, tag="ones8")
            nc.gpsimd.memset(ones8[:], 1.0)
            hqT = persist.tile([P, DH, NQ], bf16, tag="hqT")  # becomes hiddenT in place
            # half 0 of hqT lands first so Phase B's first matmul can start
            nc.sync.dma_start(hqT[:, :, 0:QT], hqT_r[:, :, 0:QT])

            hkvT8 = persist.tile([P, DH, C], fp8, tag="hkvT8")
            h_kv8 = persist.tile([P, TKV, D], fp8, tag="h_kv8")
            QTs8 = persist.tile([P, DH, NQ], fp8, tag="QTs8")
            wv_sb = persist.tile([P, DH, D], fp8, tag="wv_sb")
            mask_sb = persist.tile([P, nmask, QT], fp8, tag="mask")

            # ---- Phase B: Q' = h_q M (fp8 DoubleRow), eviction rescales ----
            with tc.tile_pool(name="qp_ps", bufs=4, space="PSUM") as qp_ps:
                for eh in range(DH):
                    wm_t = wload.tile([P, DH, P], bf16, tag="wm")
                    nc.sync.dma_start(wm_t[:], wm_r[:, :, eh * P:(eh + 1) * P])
                    if eh == 0:
                        nc.sync.dma_start(hqT[:, :, QT:NQ], hqT_r[:, :, QT:NQ])
                    for half in range(NQ // QT):
                        ps = qp_ps.tile([P, QT], f32, tag="ps")
                        for dh in range(DH):
                            nc.tensor.matmul(
                                ps[:],
                                lhsT=wm_t[:, dh, :],
                                rhs=hqT[:, dh, half * QT:(half + 1) * QT],
                                start=(dh == 0), stop=(dh == DH - 1),
                            )
                        nc.scalar.activation(
                            QTs8[:, eh, half * QT:(half + 1) * QT], ps[:],
                            Copy, scale=2.0)

            # kv-side loads enqueue after B's wm tiles so B starts early;
            # mask last (first consumed ~40us in)
            nc.sync.dma_start(hkvT8[:], hkvT8_r[:])
            nc.sync.dma_start(h_kv8[:], hkv8_r[:])
            nc.sync.dma_start(wv_sb[:], wv_r[:])
            nc.sync.dma_start(mask_sb[:], mask_d[:])

            # ---- Phase C scores: exp(QK^T) with causal mask, kb-major ----
            expm = []
            den = []
            with tc.tile_pool(name="dn_ps", bufs=2, space="PSUM") as dn_ps:
                for _qt in range(NQT):
                    expm_t = expm_pool.tile([P, NKB, QT], fp8, tag="expm_all")
                    expm.append(expm_t)
                    den_t = dn_ps.tile([P, QT], f32, tag="den")
                    den.append(den_t)
                with tc.tile_pool(name="sc_ps", bufs=4, space="PSUM") as sc_ps:
                    mi = 0
                    for kb in range(TKV):
                        for qt in range(NQT):
                            if kb >= KBM[qt]:
                                continue
                            qs = slice(qt * QT, (qt + 1) * QT)
                            s_ps = sc_ps.tile([P, QT], f32, tag="sc")
                            for dhp in range(0, DH, 2):
                                nc.tensor.matmul(
                                    s_ps[:],
                                    lhsT=hkvT8[:, dhp:dhp + 2, kb * P:(kb + 1) * P],
                                    rhs=QTs8[:, dhp:dhp + 2, qs],
                                    start=(dhp == 0), stop=(dhp == DH - 2),
                                    perf_mode=DR,
                                )
                            expT = att_sb.tile([P, QT], fp8, tag="expT")
                            # scores carry 2^21 (h 2^10 * Q' 2^11)
                            nc.scalar.activation(
                                expT[:], s_ps[:], Exp,
                                scale=float(2.0 ** -21 / np.sqrt(D)))
                            nc.vector.tensor_mul(
                                expm[qt][:, kb, :], expT[:], mask_sb[:, mi, :])
                            mi += 1
                    # softmax denominator as a tail pass (keeps the PE free
                    # running during the scores loop)
                    for qt in range(NQT):
                        for kbp in range(0, KBM[qt], 2):
                            nc.tensor.matmul(
                                den[qt][:],
                                lhsT=ones8[:],
                                rhs=expm[qt][:, kbp:kbp + 2, :],
                                start=(kbp == 0), stop=(kbp == KBM[qt] - 2),
                                perf_mode=DR,
                            )

                # ---- Phase C tail: normalize, Z = A h, attn projection ----
                with tc.tile_pool(name="zt_ps", bufs=2, space="PSUM") as zt_ps, \
                     tc.tile_pool(name="at_ps", bufs=2, space="PSUM") as at_ps:
                    for qt in range(NQT):
                        qs = slice(qt * QT, (qt + 1) * QT)
                        kbm = KBM[qt]
                        recip = att_sb.tile([P, QT], f32, tag="recip")
                        nc.vector.reciprocal(recip[:], den[qt][:])
                        ZT8 = zt_pool.tile([P, DH, QT], fp8, tag="ZT8")
                        for eh in range(DH):
                            z_ps = zt_ps.tile([P, QT], f32, tag="z")
                            for kbp in range(0, kbm, 2):
                                nc.tensor.matmul(
                                    z_ps[:],
                                    lhsT=h_kv8[:, kbp:kbp + 2, eh * P:(eh + 1) * P],
                                    rhs=expm[qt][:, kbp:kbp + 2, :],
                                    start=(kbp == 0), stop=(kbp == kbm - 2),
                                    perf_mode=DR,
                                )
                            nc.vector.tensor_mul(ZT8[:, eh, :], z_ps[:], recip[:])
                        # attn_out^T = Wv Z^T, accumulated into hiddenT (2^10)
                        for eh in range(DH):
                            a_ps = at_ps.tile([P, QT], f32, tag="at")
                            for dhp in range(0, DH, 2):
                                nc.tensor.matmul(
                                    a_ps[:],
                                    lhsT=wv_sb[:, dhp:dhp + 2, eh * P:(eh + 1) * P],
                                    rhs=ZT8[:, dhp:dhp + 2, :],
                                    start=(dhp == 0), stop=(dhp == DH - 2),
                                    perf_mode=DR,
                                )
                            tmp = att_sb.tile([P, QT], bf16, tag="tmp")
                            nc.scalar.activation(tmp[:], a_ps[:], Copy,
                                                 scale=float(2.0 ** -11))
                            nc.gpsimd.tensor_add(hqT[:, eh, qs], hqT[:, eh, qs], tmp[:])

            # ---- Phase D: logits = hiddenT^T WoT (bias added on host) ----
            nt = (V + VT - 1) // VT
            groups = []
            i = 0
            while i < nt:
                n0 = i * VT
                if i + 1 < nt:
                    groups.append([(n0, min(VT, V - n0)), (n0 + VT, min(VT, V - n0 - VT))])
                    i += 2
                else:
                    groups.append([(n0, min(VT, V - n0))])
                    i += 1

            def load_wo(grp):
                gw = sum(nw for _, nw in grp)
                g0 = grp[0][0]
                wo_c0 = wo_pool.tile([P, DH // 2, 2 * VT], bf16, tag="wo")
                wo_c1 = wo_pool.tile([P, DH // 2, 2 * VT], bf16, tag="wo")
                nc.sync.dma_start(wo_c0[:, :, :gw], woT_r[:, 0:DH // 2, g0:g0 + gw])
                nc.sync.dma_start(wo_c1[:, :, :gw], woT_r[:, DH // 2:DH, g0:g0 + gw])
                return wo_c0, wo_c1

            with tc.tile_pool(name="out_ps", bufs=8, space="PSUM") as out_ps:
                wo_cur = load_wo(groups[0])
                for gi, grp in enumerate(groups):
                    gw = sum(nw for _, nw in grp)
                    g0 = grp[0][0]
                    wo_c0, wo_c1 = wo_cur
                    if gi + 1 < len(groups):
                        wo_cur = load_wo(groups[gi + 1])
                    for m in range(TQ):
                        pss = []
                        for _j in grp:
                            ps_t = out_ps.tile([P, VT], f32, tag="out")
                            pss.append(ps_t)
                        for dh in range(DH):
                            wo_t = wo_c0 if dh < DH // 2 else wo_c1
                            for j, (n0, nw) in enumerate(grp):
                                nc.tensor.matmul(
                                    pss[j][:, :nw],
                                    lhsT=hqT[:, dh, m * P:(m + 1) * P],
                                    rhs=wo_t[:, dh % (DH // 2), j * VT:j * VT + nw],
                                    start=(dh == 0), stop=(dh == DH - 1),
                                )
                        lo = out_sb.tile([P, 2 * VT], bf16, tag="lo")
                        for j, (n0, nw) in enumerate(grp):
                            nc.scalar.activation(
                                lo[:, j * VT:j * VT + nw], pss[j][:, :nw],
                                Copy, scale=1.0)
                        nc.sync.dma_start(y_d[m * P:(m + 1) * P, g0:g0 + gw], lo[:, :gw])

    nc.compile()
    return nc


def _get_program():
    global _COMPILED
    if _COMPILED is None:
        _COMPILED = _build_program()
    return _COMPILED


def kernel(x, embed, pe, Wq, Wk, Wv, Wo, bo):
    import ml_dtypes
    from concourse.bass_utils import run_bass_kernel_spmd

    bf16 = ml_dtypes.bfloat16
    fp8 = ml_dtypes.float8_e4m3fn
    x = np.asarray(x).astype(np.int32)
    embed = np.asarray(embed, dtype=np.float32)
    pe = np.asarray(pe, dtype=np.float32)
    Wq = np.asarray(Wq, dtype=np.float32)
    Wk = np.asarray(Wk, dtype=np.float32)

    h_all = (embed[x.reshape(-1)].reshape(B, C, D) + pe[None, :C, :]) * SH
    h8_all = np.clip(h_all, -240.0, 240.0).astype(fp8)
    h_all = h_all.astype(bf16)
    wm_bf = np.ascontiguousarray((Wq.T @ Wk).astype(bf16))
    wv8 = np.ascontiguousarray(
        np.clip(np.asarray(Wv, dtype=np.float32).T * SW, -240.0, 240.0).astype(fp8))
    woT = np.ascontiguousarray(
        (np.asarray(Wo, dtype=np.float32).T * (1.0 / SH)).astype(bf16))
    bo_np = np.asarray(bo, dtype=np.float32).reshape(V)

    nc = _get_program()

    in_maps = []
    for c in range(N_CORES):
        b, hh = c // 2, c % 2
        q0 = hh * NQ
        # causal masks in (kb-major, qt-minor) loop order: mask[p, i, q] =
        # 1.0 if absolute query (q0 + qt*QT + q) >= absolute key (kb*P + p)
        blocks = []
        for kb in range(NKB):
            for qt in range(len(KBM)):
                if kb < KBM[qt]:
                    qpos = q0 + qt * QT + np.arange(QT)
                    kposv = kb * P + np.arange(P)
                    blocks.append(
                        (qpos[None, :] >= kposv[:, None]).astype(fp8))
        maskarr = np.ascontiguousarray(np.stack(blocks, axis=1))
        in_maps.append({
            "hqT": np.ascontiguousarray(h_all[b, q0:q0 + NQ].T),
            "hkvT8": np.ascontiguousarray(h8_all[b].T),
            "hkv8": h8_all[b],
            "wm": wm_bf,
            "wv": wv8,
            "woT": woT,
            "mask": maskarr,
        })

    global _last_in_maps
    _last_in_maps = in_maps
    res = run_bass_kernel_spmd(nc, in_maps, core_ids=list(range(N_CORES)))

    out = np.empty((B, C, V), dtype=np.float32)
    for c in range(N_CORES):
        b, hh = c // 2, c % 2
        out[b, hh * NQ:(hh + 1) * NQ, :] = res.results[c]["y"].astype(np.float32)
    if np.any(bo_np):
        out += bo_np[None, None, :]
    return out


# revision 6
# speedup vs baseline: 1.2872x; 1.0418x over previous
"""Trainium2 Bass kernel for nn_AbsolutePE_LM (single-head causal transformer block + LM head).

Model (fp32 reference):
    h = embed[x] + pe[:C]
    Q = h Wq^T ; K = h Wk^T ; V = h Wv^T
    A = softmax(QK^T/sqrt(D) + causal)
    hidden = h + A V
    logits = hidden Wo^T + bo

Algebraic restructure (device computes fewer FLOPs; host precomputes M):
    M := Wq^T Wk                (host, fp32)
    scores = (h_q M) h_kv^T / sqrt(D)      -> no K projection on device
    attn   = (A h_kv) Wv^T                 -> no V projection on device
    hidden = h_q + attn
    logits = hidden Wo^T       (+ bo on host)

Sharding: 8 cores = (batch b in 0..3) x (query-set s in {A,B}).  The
16 query tiles of 256 rows are split causally-balanced: set A owns
tiles {0,7,2,5}, set B owns {1,6,3,4}, so both core types need the
same per-position key-block bound BOUND=[4,16,8,14] (42 blocks, vs 56
for a contiguous halves split).  Blocks below MUL_FROM=[0,12,4,8] are
causally full for BOTH sets, so the exp() result skips the mask
multiply.  One program on all cores; per-core behaviour is carried by
the input data (gathered rows, causal masks).

Schedule: all SBUF tiles coexist (no pool scope-closes mid-kernel) so
Phase D's Wo tiles prefetch during attention.  Input DMAs are ordered
so the Q' projection (fp8 DoubleRow off hqT8/wm8) starts as soon as
~1MB lands; the bf16 hqT (residual + Phase D lhsT) streams later.
Scores run kb-pair-major: 8 DR matmuls -> one Exp over the pair ->
at most one mask multiply.  Softmax denominators accumulate in a tail
pass.  Vector evicts Q', Scalar does Exp + psum evictions, GpSimd the
residual adds.

Precision: attention matmuls run fp8(e4m3)+DoubleRow at 2x PE rate
(attn is ~2% of hidden and softmax is near-uniform); residual h and
the vocab projection in bf16; logits emitted bf16 and upcast on host.
Scales are powers of two folded into host-side tensors: embed/pe
carry 2^10, M/Wv^T carry 2^11, Wo^T carries 2^-10 so logits come out
of the last matmul unscaled.
"""

import numpy as np

V, D, MAXLEN, B, C = 32000, 1024, 2048, 4, 2048
P = 128
DH = D // P            # 8 partition tiles over the model dim
NQ = C // 2            # 1024 query rows per core
TQ = NQ // P           # 8 query row-tiles
TKV = C // P           # 16 kv row-tiles
QT = 256               # attention query-tile width
NQT = NQ // QT         # 4 attention query tiles
NKB = C // P           # 16 key blocks of 128
VT = 512               # vocab tile width
N_CORES = 8

# causally-balanced query-tile assignment (tiles of 256 rows)
TILES_A = [0, 7, 2, 5]
TILES_B = [1, 6, 3, 4]
BOUND = [2 * max(a, b) + 2 for a, b in zip(TILES_A, TILES_B)]     # [4,16,8,14]
MUL_FROM = [2 * min(a, b) for a, b in zip(TILES_A, TILES_B)]      # [0,12,4,8]
NMASKP = sum((bo - mf) // 2 for bo, mf in zip(BOUND, MUL_FROM))   # 9 pair-masks

SH = 1024.0            # 2^10 scale on h (embed/pe, host)
SW = 2048.0            # 2^11 scale on M and Wv^T (host)

_COMPILED = None


def _build_program():
    import concourse.bacc as bacc
    import concourse.mybir as mybir
    import concourse.tile as tile

    f32 = mybir.dt.float32
    bf16 = mybir.dt.bfloat16
    fp8 = mybir.dt.float8e4
    Exp = mybir.ActivationFunctionType.Exp
    Copy = mybir.ActivationFunctionType.Copy
    DR = mybir.MatmulPerfMode.DoubleRow

    nc = bacc.Bacc("TRN2", target_bir_lowering=False, debug=False, num_devices=N_CORES)

    hqT8_d = nc.dram_tensor("hqT8", [D, NQ], fp8, kind="ExternalInput").ap()
    wm8_d = nc.dram_tensor("wm8", [D, D], fp8, kind="ExternalInput").ap()
    hqT_d = nc.dram_tensor("hqT", [D, NQ], bf16, kind="ExternalInput").ap()
    hkvT8_d = nc.dram_tensor("hkvT8", [D, C], fp8, kind="ExternalInput").ap()
    hkv8_d = nc.dram_tensor("hkv8", [C, D], fp8, kind="ExternalInput").ap()
    wv_d = nc.dram_tensor("wv", [D, D], fp8, kind="ExternalInput").ap()
    woT_d = nc.dram_tensor("woT", [D, V], bf16, kind="ExternalInput").ap()
    mask_d = nc.dram_tensor("mask", [P, NMASKP, 2, QT], fp8, kind="ExternalInput").ap()
    y_d = nc.dram_tensor("y", [NQ, V], bf16, kind="ExternalOutput").ap()

    hqT8_r = hqT8_d.rearrange("(dh p) q -> p dh q", p=P)
    wm8_r = wm8_d.rearrange("(dh p) e -> p dh e", p=P)
    hqT_r = hqT_d.rearrange("(dh p) q -> p dh q", p=P)
    hkvT8_r = hkvT8_d.rearrange("(dh p) k -> p dh k", p=P)
    hkv8_r = hkv8_d.rearrange("(t p) e -> p t e", p=P)
    wv_r = wv_d.rearrange("(dh p) e -> p dh e", p=P)
    woT_r = woT_d.rearrange("(dh p) v -> p dh v", p=P)

    with tile.TileContext(nc) as tc:
        with tc.tile_pool(name="persist", bufs=1) as persist, \
             tc.tile_pool(name="att_sb", bufs=6) as att_sb, \
             tc.tile_pool(name="expm_pool", bufs=4) as expm_pool, \
             tc.tile_pool(name="zt_pool", bufs=2) as zt_pool, \
             tc.tile_pool(name="wo_pool", bufs=4) as wo_pool, \
             tc.tile_pool(name="out_sb", bufs=4) as out_sb:
            ones8 = persist.tile([P, 2, P], fp8, tag="ones8")
            nc.gpsimd.memset(ones8[:], 1.0)

            wm8 = persist.tile([P, DH, D], fp8, tag="wm8")
            hqT8 = persist.tile([P, DH, NQ], fp8, tag="hqT8")
            hqT = persist.tile([P, DH, NQ], bf16, tag="hqT")  # becomes hiddenT
            hkvT8 = persist.tile([P, DH, C], fp8, tag="hkvT8")
            h_kv8 = persist.tile([P, TKV, D], fp8, tag="h_kv8")
            QTs8 = persist.tile([P, DH, NQ], fp8, tag="QTs8")
            wv_sb = persist.tile([P, DH, D], fp8, tag="wv_sb")
            mask_sb = persist.tile([P, NMASKP, 2, QT], fp8, tag="mask")

            # staging order: Phase B inputs first, kv next, residual last
            nc.sync.dma_start(wm8[:, :, 0:D // 2], wm8_r[:, :, 0:D // 2])
            nc.sync.dma_start(hqT8[:, :, 0:NQ // 2], hqT8_r[:, :, 0:NQ // 2])
            nc.sync.dma_start(wm8[:, :, D // 2:D], wm8_r[:, :, D // 2:D])
            nc.sync.dma_start(hqT8[:, :, NQ // 2:NQ], hqT8_r[:, :, NQ // 2:NQ])
            nc.sync.dma_start(hkvT8[:], hkvT8_r[:])
            nc.sync.dma_start(h_kv8[:], hkv8_r[:])
            nc.sync.dma_start(wv_sb[:], wv_r[:])
            nc.sync.dma_start(mask_sb[:], mask_d[:])
            nc.sync.dma_start(hqT[:], hqT_r[:])

            # ---- Phase B: Q' = h_q M (fp8 DoubleRow), Vector evicts ----
            with tc.tile_pool(name="qp_ps", bufs=4, space="PSUM") as qp_ps:
                for eh in range(DH):
                    for half in range(2):
                        ps = qp_ps.tile([P, 512], f32, tag="ps")
                        for dhp in range(0, DH, 2):
                            nc.tensor.matmul(
                                ps[:],
                                lhsT=wm8[:, dhp:dhp + 2, eh * P:(eh + 1) * P],
                                rhs=hqT8[:, dhp:dhp + 2, half * 512:(half + 1) * 512],
                                start=(dhp == 0), stop=(dhp == DH - 2),
                                perf_mode=DR,
                            )
                        # ps carries 2^21 (h 2^10 * M 2^11); QTs8 carries 2^11
                        nc.vector.tensor_scalar_mul(
                            QTs8[:, eh, half * 512:(half + 1) * 512], ps[:],
                            float(2.0 ** -10))

            # ---- scores: exp(QK^T) with causal mask, kb-pair-major ----
            expm = []
            with tc.tile_pool(name="dn_ps", bufs=1, space="PSUM") as dn_ps:
                den = dn_ps.tile([P, NQT, QT], f32, tag="den")
                for _qt in range(NQT):
                    expm_t = expm_pool.tile([P, NKB, QT], fp8, tag="expm")
                    expm.append(expm_t)
                with tc.tile_pool(name="sc_ps", bufs=4, space="PSUM") as sc_ps:
                    mp = 0
                    for kbp in range(0, NKB, 2):
                        for j in range(NQT):
                            if kbp >= BOUND[j]:
                                continue
                            qs = slice(j * QT, (j + 1) * QT)
                            s_ps = sc_ps.tile([P, 2, QT], f32, tag="sc")
                            for kb in (kbp, kbp + 1):
                                for dhp in range(0, DH, 2):
                                    nc.tensor.matmul(
                                        s_ps[:, kb - kbp, :],
                                        lhsT=hkvT8[:, dhp:dhp + 2, kb * P:(kb + 1) * P],
                                        rhs=QTs8[:, dhp:dhp + 2, qs],
                                        start=(dhp == 0), stop=(dhp == DH - 2),
                                        perf_mode=DR,
                                    )
                            # scores carry 2^21 (h 2^10 * Q' 2^11)
                            if kbp >= MUL_FROM[j]:
                                expT = att_sb.tile([P, 2, QT], fp8, tag="expT")
                                nc.scalar.activation(
                                    expT[:], s_ps[:], Exp,
                                    scale=float(2.0 ** -21 / np.sqrt(D)))
                                nc.vector.tensor_mul(
                                    expm[j][:, kbp:kbp + 2, :], expT[:],
                                    mask_sb[:, mp, :, :])
                                mp += 1
                            else:
                                nc.scalar.activation(
                                    expm[j][:, kbp:kbp + 2, :], s_ps[:], Exp,
                                    scale=float(2.0 ** -21 / np.sqrt(D)))
                    assert mp == NMASKP
                    # softmax denominator tail pass
                    for j in range(NQT):
                        for kbp in range(0, BOUND[j], 2):
                            nc.tensor.matmul(
                                den[:, j, :],
                                lhsT=ones8[:],
                                rhs=expm[j][:, kbp:kbp + 2, :],
                                start=(kbp == 0), stop=(kbp == BOUND[j] - 2),
                                perf_mode=DR,
                            )

                # ---- normalize, Z = A h, attn projection, residual ----
                with tc.tile_pool(name="zt_ps", bufs=2, space="PSUM") as zt_ps, \
                     tc.tile_pool(name="at_ps", bufs=2, space="PSUM") as at_ps:
                    for j in range(NQT):
                        qs = slice(j * QT, (j + 1) * QT)
                        kbm = BOUND[j]
                        recip = att_sb.tile([P, QT], f32, tag="recip")
                        nc.vector.reciprocal(recip[:], den[:, j, :])
                        ZT8 = zt_pool.tile([P, DH, QT], fp8, tag="ZT8")
                        for eh in range(DH):
                            z_ps = zt_ps.tile([P, QT], f32, tag="z")
                            for kb2 in range(0, kbm, 2):
                                nc.tensor.matmul(
                                    z_ps[:],
                                    lhsT=h_kv8[:, kb2:kb2 + 2, eh * P:(eh + 1) * P],
                                    rhs=expm[j][:, kb2:kb2 + 2, :],
                                    start=(kb2 == 0), stop=(kb2 == kbm - 2),
                                    perf_mode=DR,
                                )
                            nc.vector.tensor_mul(ZT8[:, eh, :], z_ps[:], recip[:])
                        # attn_out^T = Wv Z^T, accumulated into hiddenT (2^10)
                        for eh in range(DH):
                            a_ps = at_ps.tile([P, QT], f32, tag="at")
                            for dhp in range(0, DH, 2):
                                nc.tensor.matmul(
                                    a_ps[:],
                                    lhsT=wv_sb[:, dhp:dhp + 2, eh * P:(eh + 1) * P],
                                    rhs=ZT8[:, dhp:dhp + 2, :],
                                    start=(dhp == 0), stop=(dhp == DH - 2),
                                    perf_mode=DR,
                                )
                            tmp = att_sb.tile([P, QT], bf16, tag="tmp")
                            nc.scalar.activation(tmp[:], a_ps[:], Copy,
                                                 scale=float(2.0 ** -11))
                            nc.gpsimd.tensor_add(hqT[:, eh, qs], hqT[:, eh, qs], tmp[:])

            # ---- Phase D: logits = hiddenT^T WoT (bias added on host) ----
            nt = (V + VT - 1) // VT
            groups = []
            i = 0
            while i < nt:
                n0 = i * VT
                if i + 1 < nt:
                    groups.append([(n0, min(VT, V - n0)), (n0 + VT, min(VT, V - n0 - VT))])
                    i += 2
                else:
                    groups.append([(n0, min(VT, V - n0))])
                    i += 1

            def load_wo(grp):
                gw = sum(nw for _, nw in grp)
                g0 = grp[0][0]
                wo_c0 = wo_pool.tile([P, DH // 2, 2 * VT], bf16, tag="wo")
                wo_c1 = wo_pool.tile([P, DH // 2, 2 * VT], bf16, tag="wo")
                nc.sync.dma_start(wo_c0[:, :, :gw], woT_r[:, 0:DH // 2, g0:g0 + gw])
                nc.sync.dma_start(wo_c1[:, :, :gw], woT_r[:, DH // 2:DH, g0:g0 + gw])
                return wo_c0, wo_c1

            with tc.tile_pool(name="out_ps", bufs=8, space="PSUM") as out_ps:
                wo_cur = load_wo(groups[0])
                for gi, grp in enumerate(groups):
                    gw = sum(nw for _, nw in grp)
                    g0 = grp[0][0]
                    wo_c0, wo_c1 = wo_cur
                    if gi + 1 < len(groups):
                        wo_cur = load_wo(groups[gi + 1])
                    for m in range(TQ):
                        pss = []
                        for _j in grp:
                            ps_t = out_ps.tile([P, VT], f32, tag="out")
                            pss.append(ps_t)
                        for dh in range(DH):
                            wo_t = wo_c0 if dh < DH // 2 else wo_c1
                            for j, (n0, nw) in enumerate(grp):
                                nc.tensor.matmul(
                                    pss[j][:, :nw],
                                    lhsT=hqT[:, dh, m * P:(m + 1) * P],
                                    rhs=wo_t[:, dh % (DH // 2), j * VT:j * VT + nw],
                                    start=(dh == 0), stop=(dh == DH - 1),
                                )
                        lo = out_sb.tile([P, 2 * VT], bf16, tag="lo")
                        for j, (n0, nw) in enumerate(grp):
                            nc.scalar.activation(
                                lo[:, j * VT:j * VT + nw], pss[j][:, :nw],
                                Copy, scale=1.0)
                        nc.sync.dma_start(y_d[m * P:(m + 1) * P, g0:g0 + gw], lo[:, :gw])

    nc.compile()
    return nc


def _get_program():
    global _COMPILED
    if _COMPILED is None:
        _COMPILED = _build_program()
    return _COMPILED


def _core_rows(hh):
    tiles = TILES_A if hh == 0 else TILES_B
    return np.concatenate([np.arange(t * QT, (t + 1) * QT) for t in tiles])


def kernel(x, embed, pe, Wq, Wk, Wv, Wo, bo):
    import ml_dtypes
    from concourse.bass_utils import run_bass_kernel_spmd

    bf16 = ml_dtypes.bfloat16
    fp8 = ml_dtypes.float8_e4m3fn
    x = np.asarray(x).astype(np.int32)
    embed = np.asarray(embed, dtype=np.float32)
    pe = np.asarray(pe, dtype=np.float32)
    Wq = np.asarray(Wq, dtype=np.float32)
    Wk = np.asarray(Wk, dtype=np.float32)

    h_all = (embed[x.reshape(-1)].reshape(B, C, D) + pe[None, :C, :]) * SH
    h8_all = np.clip(h_all, -240.0, 240.0).astype(fp8)
    h_all = h_all.astype(bf16)
    wm8 = np.ascontiguousarray(
        np.clip((Wq.T @ Wk) * SW, -240.0, 240.0).astype(fp8))
    wv8 = np.ascontiguousarray(
        np.clip(np.asarray(Wv, dtype=np.float32).T * SW, -240.0, 240.0).astype(fp8))
    woT = np.ascontiguousarray(
        (np.asarray(Wo, dtype=np.float32).T * (1.0 / SH)).astype(bf16))
    bo_np = np.asarray(bo, dtype=np.float32).reshape(V)

    nc = _get_program()

    rows = {hh: _core_rows(hh) for hh in range(2)}
    in_maps = []
    for c in range(N_CORES):
        b, hh = c // 2, c % 2
        tiles = TILES_A if hh == 0 else TILES_B
        # pair-masks in program order: kbp-major, j-minor, masked pairs only
        blocks = []
        for kbp in range(0, NKB, 2):
            for j in range(NQT):
                if kbp < BOUND[j] and kbp >= MUL_FROM[j]:
                    qpos = tiles[j] * QT + np.arange(QT)
                    kpos0 = kbp * P + np.arange(P)
                    kpos1 = (kbp + 1) * P + np.arange(P)
                    m0 = (qpos[None, :] >= kpos0[:, None]).astype(fp8)
                    m1 = (qpos[None, :] >= kpos1[:, None]).astype(fp8)
                    blocks.append(np.stack([m0, m1], axis=1))  # [P, 2, QT]
        maskarr = np.ascontiguousarray(np.stack(blocks, axis=1))  # [P, NMASKP, 2, QT]
        assert maskarr.shape == (P, NMASKP, 2, QT)
        hq = h_all[b][rows[hh]]          # [NQ, D] bf16
        hq8 = h8_all[b][rows[hh]]        # [NQ, D] fp8
        in_maps.append({
            "hqT8": np.ascontiguousarray(hq8.T),
            "wm8": wm8,
            "hqT": np.ascontiguousarray(hq.T),
            "hkvT8": np.ascontiguousarray(h8_all[b].T),
            "hkv8": h8_all[b],
            "wv": wv8,
            "woT": woT,
            "mask": maskarr,
        })

    global _last_in_maps
    _last_in_maps = in_maps
    res = run_bass_kernel_spmd(nc, in_maps, core_ids=list(range(N_CORES)))

    out = np.empty((B, C, V), dtype=np.float32)
    for c in range(N_CORES):
        b, hh = c // 2, c % 2
        out[b, rows[hh], :] = res.results[c]["y"].astype(np.float32)
    if np.any(bo_np):
        out += bo_np[None, None, :]
    return out


# revision 10
# speedup vs baseline: 1.2943x; 1.0056x over previous
"""Trainium2 Bass kernel for nn_AbsolutePE_LM (single-head causal transformer block + LM head).

Model (fp32 reference):
    h = embed[x] + pe[:C]
    Q = h Wq^T ; K = h Wk^T ; V = h Wv^T
    A = softmax(QK^T/sqrt(D) + causal)
    hidden = h + A V
    logits = hidden Wo^T + bo

Algebraic restructure (device computes fewer FLOPs; host precomputes M):
    M := Wq^T Wk                (host, fp32)
    scores = (h_q M) h_kv^T / sqrt(D)      -> no K projection on device
    attn   = (A h_kv) Wv^T                 -> no V projection on device
    hidden = h_q + attn
    logits = hidden Wo^T       (+ bo on host)

Sharding: 8 cores = (batch b in 0..3) x (query-set s in {A,B}).  The
16 query tiles of 256 rows are split causally-balanced: set A owns
tiles {0,7,2,5}, set B owns {1,6,3,4}, so both core types need the
same per-position key-block bound BOUND=[4,16,8,14] (42 blocks, vs 56
for a contiguous halves split).  Blocks below MUL_FROM=[0,12,4,8] are
causally full for BOTH sets, so the exp() result skips the mask
multiply.  One program on all cores; per-core behaviour is carried by
the input data (gathered rows, causal masks).

Schedule: attention runs query-tile-major (j-major) so hidden rows
j0/j1 finish early; Phase D then starts on row-half m0..3 for the
first KA vocab groups (re-streaming those Wo tiles once more for the
m4..7 catch-up pass) while the j2/j3 attention tail hides under the
Phase D matmul stream.  All SBUF tiles coexist; PSUM pools are
managed manually to fit the 8 banks per phase.  Input DMAs are
ordered along the attention critical path (Q' inputs, keys, masks,
kv-rows, Wv, residual halves, then the Wo stream).  Vector+GpSimd
split the Q' evictions and residual adds; Scalar does Exp and all
Phase D psum evictions.

Precision: attention matmuls run fp8(e4m3)+DoubleRow at 2x PE rate
(attn is ~2% of hidden and softmax is near-uniform); residual h and
the vocab projection in bf16; logits emitted bf16 and upcast on host.
Scales are powers of two folded into host-side tensors: embed/pe
carry 2^10, M/Wv^T carry 2^11, Wo^T carries 2^-10 so logits come out
of the last matmul unscaled.
"""

import numpy as np

V, D, MAXLEN, B, C = 32000, 1024, 2048, 4, 2048
P = 128
DH = D // P            # 8 partition tiles over the model dim
NQ = C // 2            # 1024 query rows per core
TQ = NQ // P           # 8 query row-tiles
TKV = C // P           # 16 kv row-tiles
QT = 256               # attention query-tile width
NQT = NQ // QT         # 4 attention query tiles
NKB = C // P           # 16 key blocks of 128
VT = 512               # vocab tile width
N_CORES = 8
KA = 4                 # vocab groups run split m0..3 / m4..7 to hide attention

# causally-balanced query-tile assignment (tiles of 256 rows)
TILES_A = [0, 7, 2, 5]
TILES_B = [1, 6, 3, 4]
BOUND = [2 * max(a, b) + 2 for a, b in zip(TILES_A, TILES_B)]     # [4,16,8,14]
MUL_FROM = [2 * min(a, b) for a, b in zip(TILES_A, TILES_B)]      # [0,12,4,8]
NMASKP = sum((bo - mf) // 2 for bo, mf in zip(BOUND, MUL_FROM))   # 9 pair-masks

SH = 1024.0            # 2^10 scale on h (embed/pe, host)
SW = 2048.0            # 2^11 scale on M and Wv^T (host)

_COMPILED = None


def _build_program():
    import concourse.bacc as bacc
    import concourse.mybir as mybir
    import concourse.tile as tile

    f32 = mybir.dt.float32
    bf16 = mybir.dt.bfloat16
    fp8 = mybir.dt.float8e4
    Exp = mybir.ActivationFunctionType.Exp
    Copy = mybir.ActivationFunctionType.Copy
    DR = mybir.MatmulPerfMode.DoubleRow

    nc = bacc.Bacc("TRN2", target_bir_lowering=False, debug=False, num_devices=N_CORES)

    hqT8_d = nc.dram_tensor("hqT8", [D, NQ], fp8, kind="ExternalInput").ap()
    wm8_d = nc.dram_tensor("wm8", [D, D], fp8, kind="ExternalInput").ap()
    hqT_d = nc.dram_tensor("hqT", [D, NQ], bf16, kind="ExternalInput").ap()
    hkvT8_d = nc.dram_tensor("hkvT8", [D, C], fp8, kind="ExternalInput").ap()
    hkv8_d = nc.dram_tensor("hkv8", [C, D], fp8, kind="ExternalInput").ap()
    wv_d = nc.dram_tensor("wv", [D, D], fp8, kind="ExternalInput").ap()
    woT_d = nc.dram_tensor("woT", [D, V], bf16, kind="ExternalInput").ap()
    mask_d = nc.dram_tensor("mask", [P, NMASKP, 2, QT], fp8, kind="ExternalInput").ap()
    y_d = nc.dram_tensor("y", [NQ, V], bf16, kind="ExternalOutput").ap()

    hqT8_r = hqT8_d.rearrange("(dh p) q -> p dh q", p=P)
    wm8_r = wm8_d.rearrange("(dh p) e -> p dh e", p=P)
    hqT_r = hqT_d.rearrange("(dh p) q -> p dh q", p=P)
    hkvT8_r = hkvT8_d.rearrange("(dh p) k -> p dh k", p=P)
    hkv8_r = hkv8_d.rearrange("(t p) e -> p t e", p=P)
    wv_r = wv_d.rearrange("(dh p) e -> p dh e", p=P)
    woT_r = woT_d.rearrange("(dh p) v -> p dh v", p=P)

    with tile.TileContext(nc) as tc:
        with tc.tile_pool(name="persist", bufs=1) as persist, \
             tc.tile_pool(name="att_sb", bufs=6) as att_sb, \
             tc.tile_pool(name="recip_sb", bufs=4) as recip_sb, \
             tc.tile_pool(name="expm_pool", bufs=4) as expm_pool, \
             tc.tile_pool(name="zt_pool", bufs=2) as zt_pool, \
             tc.tile_pool(name="wo_pool", bufs=4) as wo_pool, \
             tc.tile_pool(name="out_sb", bufs=4) as out_sb:
            ones8 = persist.tile([P, 2, P], fp8, tag="ones8")
            nc.gpsimd.memset(ones8[:], 1.0)

            wm8 = persist.tile([P, DH, D], fp8, tag="wm8")
            hqT8 = persist.tile([P, DH, NQ], fp8, tag="hqT8")
            hqT = persist.tile([P, DH, NQ], bf16, tag="hqT")  # becomes hiddenT
            hkvT8 = persist.tile([P, DH, C], fp8, tag="hkvT8")
            h_kv8 = persist.tile([P, TKV, D], fp8, tag="h_kv8")
            QTs8 = persist.tile([P, DH, NQ], fp8, tag="QTs8")
            wv_sb = persist.tile([P, DH, D], fp8, tag="wv_sb")
            mask_sb = persist.tile([P, NMASKP, 2, QT], fp8, tag="mask")

            # staging order = attention critical path
            nc.sync.dma_start(wm8[:], wm8_r[:])
            nc.sync.dma_start(hqT8[:], hqT8_r[:])
            nc.sync.dma_start(hkvT8[:], hkvT8_r[:])
            nc.sync.dma_start(mask_sb[:], mask_d[:])
            nc.sync.dma_start(wv_sb[:], wv_r[:])
            nc.sync.dma_start(h_kv8[:, 0:4, :], hkv8_r[:, 0:4, :])
            nc.sync.dma_start(hqT[:, :, 0:2 * QT], hqT_r[:, :, 0:2 * QT])
            nc.sync.dma_start(h_kv8[:, 4:TKV, :], hkv8_r[:, 4:TKV, :])
            nc.sync.dma_start(hqT[:, :, 2 * QT:NQ], hqT_r[:, :, 2 * QT:NQ])

            # ---- Phase B: Q' = h_q M (fp8 DoubleRow) ----
            with tc.tile_pool(name="qp_ps", bufs=4, space="PSUM") as qp_ps:
                for eh in range(DH):
                    for half in range(2):
                        ps = qp_ps.tile([P, 512], f32, tag="ps")
                        for dhp in range(0, DH, 2):
                            nc.tensor.matmul(
                                ps[:],
                                lhsT=wm8[:, dhp:dhp + 2, eh * P:(eh + 1) * P],
                                rhs=hqT8[:, dhp:dhp + 2, half * 512:(half + 1) * 512],
                                start=(dhp == 0), stop=(dhp == DH - 2),
                                perf_mode=DR,
                            )
                        # ps carries 2^21 (h 2^10 * M 2^11); QTs8 carries 2^11
                        if eh % 2 == 1:
                            nc.scalar.activation(
                                QTs8[:, eh, half * 512:(half + 1) * 512], ps[:],
                                Copy, scale=float(2.0 ** -10))
                        else:
                            nc.vector.tensor_scalar_mul(
                                QTs8[:, eh, half * 512:(half + 1) * 512], ps[:],
                                float(2.0 ** -10))

            # ---- attention, query-tile-major ----
            dn_ps = tc.alloc_tile_pool(name="dn_ps", bufs=1, space="PSUM")
            sc_ps = tc.alloc_tile_pool(name="sc_ps", bufs=2, space="PSUM")
            den = dn_ps.tile([P, NQT, QT], f32, tag="den")
            expm = []
            recips = []
            for _qt in range(NQT):
                expm_t = expm_pool.tile([P, NKB, QT], fp8, tag="expm")
                expm.append(expm_t)
                recip_t = recip_sb.tile([P, QT], f32, tag="recip")
                recips.append(recip_t)
            mask_order = []   # host must build masks in this order
            for j in range(NQT):
                for kbp in range(MUL_FROM[j], BOUND[j], 2):
                    mask_order.append((j, kbp))

            def scores_j(j):
                qs = slice(j * QT, (j + 1) * QT)
                for kbp in range(0, BOUND[j], 2):
                    s_ps = sc_ps.tile([P, 2, QT], f32, tag="sc")
                    for kb in (kbp, kbp + 1):
                        for dhp in range(0, DH, 2):
                            nc.tensor.matmul(
                                s_ps[:, kb - kbp, :],
                                lhsT=hkvT8[:, dhp:dhp + 2, kb * P:(kb + 1) * P],
                                rhs=QTs8[:, dhp:dhp + 2, qs],
                                start=(dhp == 0), stop=(dhp == DH - 2),
                                perf_mode=DR,
                            )
                    # scores carry 2^21 (h 2^10 * Q' 2^11)
                    if kbp >= MUL_FROM[j]:
                        expT = att_sb.tile([P, 2, QT], fp8, tag="expT")
                        nc.scalar.activation(
                            expT[:], s_ps[:], Exp,
                            scale=float(2.0 ** -21 / np.sqrt(D)))
                        mp = mask_order.index((j, kbp))
                        nc.vector.tensor_mul(
                            expm[j][:, kbp:kbp + 2, :], expT[:],
                            mask_sb[:, mp, :, :])
                    else:
                        nc.scalar.activation(
                            expm[j][:, kbp:kbp + 2, :], s_ps[:], Exp,
                            scale=float(2.0 ** -21 / np.sqrt(D)))
                for kbp in range(0, BOUND[j], 2):
                    nc.tensor.matmul(
                        den[:, j, :],
                        lhsT=ones8[:],
                        rhs=expm[j][:, kbp:kbp + 2, :],
                        start=(kbp == 0), stop=(kbp == BOUND[j] - 2),
                        perf_mode=DR,
                    )
                nc.vector.reciprocal(recips[j][:], den[:, j, :])

            def ctail_j(j, zt_ps, at_ps):
                qs = slice(j * QT, (j + 1) * QT)
                kbm = BOUND[j]
                ZT8 = zt_pool.tile([P, DH, QT], fp8, tag="ZT8")
                for eh in range(DH):
                    z_ps = zt_ps.tile([P, QT], f32, tag="z")
                    for kb2 in range(0, kbm, 2):
                        nc.tensor.matmul(
                            z_ps[:],
                            lhsT=h_kv8[:, kb2:kb2 + 2, eh * P:(eh + 1) * P],
                            rhs=expm[j][:, kb2:kb2 + 2, :],
                            start=(kb2 == 0), stop=(kb2 == kbm - 2),
                            perf_mode=DR,
                        )
                    nc.vector.tensor_mul(ZT8[:, eh, :], z_ps[:], recips[j][:])
                # attn_out^T = Wv Z^T, accumulated into hiddenT (2^10)
                for eh in range(DH):
                    a_ps = at_ps.tile([P, QT], f32, tag="at")
                    for dhp in range(0, DH, 2):
                        nc.tensor.matmul(
                            a_ps[:],
                            lhsT=wv_sb[:, dhp:dhp + 2, eh * P:(eh + 1) * P],
                            rhs=ZT8[:, dhp:dhp + 2, :],
                            start=(dhp == 0), stop=(dhp == DH - 2),
                            perf_mode=DR,
                        )
                    tmp = att_sb.tile([P, QT], bf16, tag="tmp")
                    nc.scalar.activation(tmp[:], a_ps[:], Copy,
                                         scale=float(2.0 ** -11))
                    nc.gpsimd.tensor_add(hqT[:, eh, qs], hqT[:, eh, qs], tmp[:])

            scores_j(0)
            scores_j(1)
            zt_ps = tc.alloc_tile_pool(name="zt_ps", bufs=2, space="PSUM", side="right")
            at_ps = tc.alloc_tile_pool(name="at_ps", bufs=2, space="PSUM", side="right")
            ctail_j(0, zt_ps, at_ps)
            ctail_j(1, zt_ps, at_ps)
            scores_j(2)
            scores_j(3)
            sc_ps.release()
            dn_ps.release()

            # ---- Phase D: logits = hiddenT^T WoT (bias added on host) ----
            nt = (V + VT - 1) // VT
            groups = []
            i = 0
            while i < nt:
                n0 = i * VT
                if i + 1 < nt:
                    groups.append([(n0, min(VT, V - n0)), (n0 + VT, min(VT, V - n0 - VT))])
                    i += 2
                else:
                    groups.append([(n0, min(VT, V - n0))])
                    i += 1

            # jobs: (group, m_lo, m_hi); first KA groups run split so the
            # m0..3 half starts as soon as j0/j1 hidden rows are ready
            jobs = [(g, 0, TQ // 2) for g in groups[:KA]] \
                 + [(g, TQ // 2, TQ) for g in groups[:KA]] \
                 + [(g, 0, TQ) for g in groups[KA:]]

            def load_wo(grp):
                gw = sum(nw for _, nw in grp)
                g0 = grp[0][0]
                wo_c0 = wo_pool.tile([P, DH // 2, 2 * VT], bf16, tag="wo")
                wo_c1 = wo_pool.tile([P, DH // 2, 2 * VT], bf16, tag="wo")
                nc.sync.dma_start(wo_c0[:, :, :gw], woT_r[:, 0:DH // 2, g0:g0 + gw])
                nc.sync.dma_start(wo_c1[:, :, :gw], woT_r[:, DH // 2:DH, g0:g0 + gw])
                return wo_c0, wo_c1

            out_ps = tc.alloc_tile_pool(name="out_ps1", bufs=4, space="PSUM")
            wo_cur = load_wo(jobs[0][0])
            for ji, (grp, m_lo, m_hi) in enumerate(jobs):
                gw = sum(nw for _, nw in grp)
                g0 = grp[0][0]
                wo_c0, wo_c1 = wo_cur
                if ji + 1 < len(jobs):
                    wo_cur = load_wo(jobs[ji + 1][0])
                if ji == 2 * KA:
                    # all split jobs done: attention tail is retired, swap
                    # to the full-width psum pool
                    at_ps.release()
                    zt_ps.release()
                    out_ps.release()
                    out_ps = tc.alloc_tile_pool(name="out_ps2", bufs=8, space="PSUM")
                for m in range(m_lo, m_hi):
                    pss = []
                    for _j in grp:
                        ps_t = out_ps.tile([P, VT], f32, tag="out")
                        pss.append(ps_t)
                    for dh in range(DH):
                        wo_t = wo_c0 if dh < DH // 2 else wo_c1
                        for j, (n0, nw) in enumerate(grp):
                            nc.tensor.matmul(
                                pss[j][:, :nw],
                                lhsT=hqT[:, dh, m * P:(m + 1) * P],
                                rhs=wo_t[:, dh % (DH // 2), j * VT:j * VT + nw],
                                start=(dh == 0), stop=(dh == DH - 1),
                            )
                    lo = out_sb.tile([P, 2 * VT], bf16, tag="lo")
                    for j, (n0, nw) in enumerate(grp):
                        nc.scalar.activation(
                            lo[:, j * VT:j * VT + nw], pss[j][:, :nw],
                            Copy, scale=1.0)
                    nc.sync.dma_start(y_d[m * P:(m + 1) * P, g0:g0 + gw], lo[:, :gw])
                # attention tail interleaves with the first split jobs
                if ji == 0:
                    ctail_j(2, zt_ps, at_ps)
                elif ji == 1:
                    ctail_j(3, zt_ps, at_ps)
            out_ps.release()

    nc.compile()
    return nc


def _get_program():
    global _COMPILED
    if _COMPILED is None:
        _COMPILED = _build_program()
    return _COMPILED


def _core_rows(hh):
    tiles = TILES_A if hh == 0 else TILES_B
    return np.concatenate([np.arange(t * QT, (t + 1) * QT) for t in tiles])


def kernel(x, embed, pe, Wq, Wk, Wv, Wo, bo):
    import ml_dtypes
    from concourse.bass_utils import run_bass_kernel_spmd

    bf16 = ml_dtypes.bfloat16
    fp8 = ml_dtypes.float8_e4m3fn
    x = np.asarray(x).astype(np.int32)
    embed = np.asarray(embed, dtype=np.float32)
    pe = np.asarray(pe, dtype=np.float32)
    Wq = np.asarray(Wq, dtype=np.float32)
    Wk = np.asarray(Wk, dtype=np.float32)

    h_all = (embed[x.reshape(-1)].reshape(B, C, D) + pe[None, :C, :]) * SH
    h8_all = np.clip(h_all, -240.0, 240.0).astype(fp8)
    h_all = h_all.astype(bf16)
    wm8 = np.ascontiguousarray(
        np.clip((Wq.T @ Wk) * SW, -240.0, 240.0).astype(fp8))
    wv8 = np.ascontiguousarray(
        np.clip(np.asarray(Wv, dtype=np.float32).T * SW, -240.0, 240.0).astype(fp8))
    woT = np.ascontiguousarray(
        (np.asarray(Wo, dtype=np.float32).T * (1.0 / SH)).astype(bf16))
    bo_np = np.asarray(bo, dtype=np.float32).reshape(V)

    nc = _get_program()

    rows = {hh: _core_rows(hh) for hh in range(2)}
    in_maps = []
    for c in range(N_CORES):
        b, hh = c // 2, c % 2
        tiles = TILES_A if hh == 0 else TILES_B
        # pair-masks in program order: j-major, masked pairs only
        blocks = []
        for j in range(NQT):
            for kbp in range(MUL_FROM[j], BOUND[j], 2):
                qpos = tiles[j] * QT + np.arange(QT)
                kpos0 = kbp * P + np.arange(P)
                kpos1 = (kbp + 1) * P + np.arange(P)
                m0 = (qpos[None, :] >= kpos0[:, None]).astype(fp8)
                m1 = (qpos[None, :] >= kpos1[:, None]).astype(fp8)
                blocks.append(np.stack([m0, m1], axis=1))  # [P, 2, QT]
        maskarr = np.ascontiguousarray(np.stack(blocks, axis=1))  # [P, NMASKP, 2, QT]
        assert maskarr.shape == (P, NMASKP, 2, QT)
        hq = h_all[b][rows[hh]]          # [NQ, D] bf16
        hq8 = h8_all[b][rows[hh]]        # [NQ, D] fp8
        in_maps.append({
            "hqT8": np.ascontiguousarray(hq8.T),
            "wm8": wm8,
            "hqT": np.ascontiguousarray(hq.T),
            "hkvT8": np.ascontiguousarray(h8_all[b].T),
            "hkv8": h8_all[b],
            "wv": wv8,
            "woT": woT,
            "mask": maskarr,
        })

    global _last_in_maps
    _last_in_maps = in_maps
    res = run_bass_kernel_spmd(nc, in_maps, core_ids=list(range(N_CORES)))

    out = np.empty((B, C, V), dtype=np.float32)
    for c in range(N_CORES):
        b, hh = c // 2, c % 2
        out[b, rows[hh], :] = res.results[c]["y"].astype(np.float32)
    if np.any(bo_np):
        out += bo_np[None, None, :]
    return out
